# revision 1
# baseline (speedup 1.0000x reference)
"""Trainium2 Bass kernel for nn_AttentionRefinementModule (deformable conv + sigmoid).

Data-parallel over batch: 8 samples -> 8 NeuronCores.

Changes vs v1 baseline:
  - Pair-row gather: per (tap k, position p) TWO dma_gather descriptors
    (one per y-corner row), each fetching the contiguous x-corner pair
    [2 x 256 o] from a per-k UT region -> half the SWDGE descriptor-
    generation time on GpSimd (the v1 bottleneck).
  - UT stored per-k: ut_d[k][q, 256] bf16 so the x-pair (q, q+1) is one
    contiguous 1KB element (elem_step=256, overlapping elements).
  - P5 weighted-reduce restructured into tensor_tensor adds on contiguous
    bf16 slices (4x DVE mode) instead of strided tensor_reduce (1x mode).
"""

import numpy as np

import concourse.bass as bass
import concourse.mybir as mybir
from concourse import bacc
from concourse.tile import TileContext
from concourse.bass_utils import run_bass_kernel_spmd

B, C, H, W = 8, 256, 64, 64
HW = H * W
NK = 9
PW = W + 2           # 66
NPAD = 4800          # xpad free size per channel-half
NT = HW // 128       # 32 interior position tiles
NT2 = 35             # padded-grid tiles (35*128 = 4480)
NQ = NT2 * 128       # 4480 UT rows per k
NG = 16              # gather groups (2 tiles each)
F32 = mybir.dt.float32
BF16 = mybir.dt.bfloat16
I32 = mybir.dt.int32
I16 = mybir.dt.int16
FP8 = mybir.dt.float8e4

_CACHE = {}


def build_nc(debug=False):
    nc = bacc.Bacc()

    x_d = nc.declare_dram_parameter("x", [C, HW], F32, isOutput=False)
    wofflhsT_d = nc.declare_dram_parameter("wofflhsT", [128, 2, NK, 18], BF16, isOutput=False)
    wmov_d = nc.declare_dram_parameter("wmov", [128, 2, NK * 256], BF16, isOutput=False)
    boff_d = nc.declare_dram_parameter("boff", [18, 1], F32, isOutput=False)
    bias_d = nc.declare_dram_parameter("bias", [128, 2], F32, isOutput=False)
    ident_d = nc.declare_dram_parameter("ident", [128, 128], F32, isOutput=False)
    basey_d = nc.declare_dram_parameter("basey", [128, NT], F32, isOutput=False)
    basex_d = nc.declare_dram_parameter("basex", [128, 1], F32, isOutput=False)
    kty_d = nc.declare_dram_parameter("kty", [128, NK], F32, isOutput=False)
    ktx_d = nc.declare_dram_parameter("ktx", [128, NK], F32, isOutput=False)
    sel_d = nc.declare_dram_parameter("sel", [128, 8, 16], F32, isOutput=False)
    out_d = nc.declare_dram_parameter("out", [C, HW], F32, isOutput=True)
    if debug:
        dbg_wgt = nc.declare_dram_parameter("dbg_wgt", [128, NG * NK * 8], F32, isOutput=True)
        dbg_idxw = nc.declare_dram_parameter("dbg_idxw", [128, NK * NG * 32], I16, isOutput=True)
        dbg_outT = nc.declare_dram_parameter("dbg_outT", [128, NT * 256], F32, isOutput=True)

    with TileContext(nc) as tc:
        free_order = []
        free_fns = {}

        def single(name, shape, dt=F32):
            t, fr = tc.tile(shape, dt, name=name)
            free_fns[name] = fr
            free_order.append(name)
            return t

        with (
            tc.tile_pool(name="psoff", bufs=1, space="PSUM") as ps_off,
            tc.tile_pool(name="pstr", bufs=2, space="PSUM") as ps_tr,
            tc.tile_pool(name="psut", bufs=1, space="PSUM") as ps_ut,
            tc.tile_pool(name="dram", bufs=1, space="DRAM") as dpool,
        ):
            ut_d = dpool.tile([NK, NQ * 256], FP8, name="ut")

            # ---- persistent tiles ----
            # wg2[p, grp, k, 4*tt + (2*r + xc)] = corner weight
            wg2 = single("wg2", [128, NG, NK, 8])
            wg2b = single("wg2b", [128, NG, NK, 8], BF16)
            # idxw2[p, k, grp, a + 8*r + 16*tt] wrapped int16 row-pair idx
            idxw2 = single("idxw2", [128, NK, NG, 32], I16)
            outT = single("outT", [128, NT, 256])
            ident = single("ident", [128, 128])
            nc.sync.dma_start(out=ident[:, :], in_=ident_d[:, :])
            identb = single("identb", [128, 128], BF16)
            nc.vector.tensor_copy(identb[:, :], ident[:, :])
            woff_sb = single("woff_sb", [128, 2, NK, 18], BF16)
            nc.sync.dma_start(out=woff_sb[:, :, :, :], in_=wofflhsT_d[:, :, :, :])
            wmov_sb = single("wmov_sb", [128, 2, NK * 256], BF16)
            nc.sync.dma_start(out=wmov_sb[:, :, :], in_=wmov_d[:, :, :])
            boff_sb = single("boff_sb", [18, 1])
            nc.sync.dma_start(out=boff_sb[:, :], in_=boff_d[:, :])
            bias_sb = single("bias_sb", [128, 2])
            nc.sync.dma_start(out=bias_sb[:, :], in_=bias_d[:, :])
            xpad = single("xpad", [128, 2, NPAD], BF16)

            # ---- freeable constants ----
            basey = single("basey", [128, NT])
            nc.sync.dma_start(out=basey[:, :], in_=basey_d[:, :])
            basex = single("basex", [128, 1])
            nc.sync.dma_start(out=basex[:, :], in_=basex_d[:, :])
            kty = single("kty", [128, NK])
            nc.sync.dma_start(out=kty[:, :], in_=kty_d[:, :])
            ktx = single("ktx", [128, NK])
            nc.sync.dma_start(out=ktx[:, :], in_=ktx_d[:, :])
            sel = single("sel", [128, 8, 16])
            nc.sync.dma_start(out=sel[:, :, :], in_=sel_d[:, :, :])

            # ---- P0: padded bf16 input image ----
            nc.vector.memset(xpad[:, :, :], 0.0)
            for g in range(2):
                dst = bass.AP(xpad.tensor, xpad.offset + g * NPAD + PW + 1,
                              [xpad.ap[0], [PW, H], [1, W]])
                nc.gpsimd.dma_start(
                    out=dst,
                    in_=x_d[g * 128:(g + 1) * 128, :].rearrange(
                        "c (h w) -> c h w", w=W))

            # ---- P1: offset conv on the padded grid (flat shifted views) ----
            offp_sb = single("offp_sb", [18, 4608])
            off_sb = single("off_sb", [18, HW])
            for n in range(9):
                ps = ps_off.tile([18, 512], F32, name="ps_off_t")
                first = True
                for g in range(2):
                    for t in range(NK):
                        ty, tx = t // 3, t % 3
                        o0 = ty * PW + tx + n * 512
                        rhs = xpad[:, g, o0:o0 + 512]
                        nc.tensor.matmul(ps[:, :], woff_sb[:, g, t, :], rhs,
                                         start=first,
                                         stop=(g == 1 and t == NK - 1))
                        first = False
                nc.vector.tensor_scalar(offp_sb[:, n * 512:(n + 1) * 512],
                                        ps[:, :], boff_sb[:, :], None,
                                        mybir.AluOpType.add)
            nc.vector.tensor_copy(
                off_sb.rearrange("j (h w) -> j h w", w=W),
                bass.AP(offp_sb.tensor, offp_sb.offset,
                        [offp_sb.ap[0], [PW, H], [1, W]]))

            # ---- P2: transpose off -> offT [128(hw%128), 32(t), 18] ----
            offT = single("offT", [128, NT, 18])
            for t in range(NT):
                pst = ps_tr.tile([128, 128], F32, name="ps_tr_t")
                nc.tensor.transpose(pst[:, :18],
                                    off_sb[:, t * 128:(t + 1) * 128],
                                    ident[:18, :18])
                nc.vector.tensor_copy(offT[:, t, :], pst[:, :18])

            # ---- P3: corner weights + row-pair gather indices ----
            SH = [128, NT, NK]

            def bcast_tk(ap_pk):
                return bass.AP(ap_pk.tensor, ap_pk.offset,
                               [ap_pk.ap[0], [0, NT], ap_pk.ap[1]])

            def bcast_pt(ap_pt):
                return bass.AP(ap_pt.tensor, ap_pt.offset,
                               [ap_pt.ap[0], ap_pt.ap[1], [0, NK]])

            dyx = offT.rearrange("p t (k two) -> p two t k", two=2)
            dy, dx = dyx[:, 0], dyx[:, 1]

            py = single("py", SH)
            px = single("px", SH)
            tA = single("tA", SH)
            tB = single("tB", SH)
            nc.vector.tensor_add(tA[:, :, :], dy, bcast_tk(kty[:, :]))
            nc.vector.tensor_add(py[:, :, :], tA[:, :, :], bcast_pt(basey[:, :]))
            nc.vector.tensor_add(tB[:, :, :], dx, bcast_tk(ktx[:, :]))
            nc.vector.tensor_add(px[:, :, :], tB[:, :, :],
                                 bass.AP(basex.tensor, basex.offset,
                                         [basex.ap[0], [0, NT], [0, NK]]))

            def floor_split(p_ap, nm):
                t16 = single(nm + "_t16", SH)
                nc.vector.tensor_scalar_add(t16[:, :, :], p_ap, 16.0)
                ti = single(nm + "_ti", SH, I32)
                nc.vector.tensor_copy(ti[:, :, :], t16[:, :, :])
                tif = single(nm + "_tif", SH)
                nc.vector.tensor_copy(tif[:, :, :], ti[:, :, :])
                fr = single(nm + "_fr", SH)
                nc.vector.tensor_sub(fr[:, :, :], t16[:, :, :], tif[:, :, :])
                ng = single(nm + "_ng", SH)
                nc.vector.tensor_scalar(ng[:, :, :], fr[:, :, :], 0.0, None,
                                        mybir.AluOpType.is_lt)
                w1 = single(nm + "_w1", SH)
                nc.vector.tensor_add(w1[:, :, :], fr[:, :, :], ng[:, :, :])
                t2 = single(nm + "_t2", SH)
                nc.vector.tensor_sub(t2[:, :, :], tif[:, :, :], ng[:, :, :])
                f0 = single(nm + "_f0", SH)
                nc.vector.tensor_scalar_sub(f0[:, :, :], t2[:, :, :], 16.0)
                return f0, w1

            y0, wy1 = floor_split(py[:, :, :], "y")
            x0, wx1 = floor_split(px[:, :, :], "x")
            wy0 = single("wy0", SH)
            nc.vector.tensor_scalar(wy0[:, :, :], wy1[:, :, :], -1.0, 1.0,
                                    mybir.AluOpType.mult, mybir.AluOpType.add)
            wx0 = single("wx0", SH)
            nc.vector.tensor_scalar(wx0[:, :, :], wx1[:, :, :], -1.0, 1.0,
                                    mybir.AluOpType.mult, mybir.AluOpType.add)
            y1 = single("y1", SH)
            nc.vector.tensor_scalar_add(y1[:, :, :], y0[:, :, :], 1.0)
            x1 = single("x1", SH)
            nc.vector.tensor_scalar_add(x1[:, :, :], x0[:, :, :], 1.0)

            def valid_w(cf, wraw, lim, nm):
                v0 = single(nm + "_v0", SH)
                nc.vector.tensor_scalar(v0[:, :, :], cf[:, :, :], 0.0, None,
                                        mybir.AluOpType.is_ge)
                v1 = single(nm + "_v1", SH)
                nc.vector.tensor_scalar(v1[:, :, :], cf[:, :, :], float(lim),
                                        None, mybir.AluOpType.is_le)
                wv = single(nm + "_wv", SH)
                nc.vector.tensor_mul(wv[:, :, :], v0[:, :, :], v1[:, :, :])
                wc = single(nm + "_wc", SH)
                nc.vector.tensor_mul(wc[:, :, :], wraw[:, :, :], wv[:, :, :])
                return wc

            wy0c = valid_w(y0, wy0, H - 1, "cy0")
            wy1c = valid_w(y1, wy1, H - 1, "cy1")
            wx0c = valid_w(x0, wx0, W - 1, "cx0")
            wx1c = valid_w(x1, wx1, W - 1, "cx1")

            # weights into wg2[p, grp, k, 4*tt + 2*r + xc]
            def gkt_view(ap3):
                # [128, NT, NK] (strides t:NK, k:1) viewed as (grp, k, tt)
                return bass.AP(ap3.tensor, ap3.offset,
                               [ap3.ap[0], [2 * NK, NG], [1, NK], [NK, 2]])

            for r, wyc in enumerate([wy0c, wy1c]):
                for xc, wxc in enumerate([wx0c, wx1c]):
                    m = 2 * r + xc
                    wdst = bass.AP(wg2.tensor, wg2.offset + m,
                                   [wg2.ap[0], [NK * 8, NG], [8, NK], [4, 2]])
                    nc.vector.tensor_mul(wdst, gkt_view(wyc), gkt_view(wxc))

            # gather row-pair indices: q_r = clamp(y0+1+r,0,65)*66 + clamp(x0+1,0,64)
            xq = single("xq", SH)
            nc.vector.tensor_scalar(xq[:, :, :], x0[:, :, :], 1.0, 0.0,
                                    mybir.AluOpType.add, mybir.AluOpType.max)
            nc.vector.tensor_scalar_min(xq[:, :, :], xq[:, :, :], 64.0)
            idxq2 = single("idxq2", [128, NK, NT, 2])
            for r in range(2):
                yq = single(f"yq{r}", SH)
                nc.vector.tensor_scalar(yq[:, :, :], y0[:, :, :],
                                        float(1 + r), 0.0,
                                        mybir.AluOpType.add,
                                        mybir.AluOpType.max)
                nc.vector.tensor_scalar_min(yq[:, :, :], yq[:, :, :], 65.0)
                nc.vector.tensor_scalar_mul(yq[:, :, :], yq[:, :, :], 66.0)
                # idxq2[p, k, t, r] = yq*66 + xq  (iteration (k, t))
                nc.vector.tensor_add(
                    idxq2[:, :, :, r],
                    yq.rearrange("p t k -> p k t"),
                    xq.rearrange("p t k -> p k t"))

            # ---- P3b: fold idxq2 into wrapped int16 gather layout ----
            # i = p + 128*r + 256*tt  (per (k, grp) list of 512)
            # -> idxw2[c, k, grp, a + 8*r + 16*tt], p = 16a + c
            for k in range(NK):
                for a in range(8):
                    psq = ps_tr.tile([128, 128], F32, name="ps_tr_t")
                    nc.tensor.matmul(psq[:16, :64], sel[:, a, :],
                                     idxq2[:, k, :, :].rearrange(
                                         "p t r -> p (t r)"),
                                     start=True, stop=True)
                    # src col j = 4g + 2tt + r ; dst col = a + 8r + 16tt
                    dst = bass.AP(idxw2.tensor,
                                  idxw2.offset + k * (NG * 32) + a,
                                  [[idxw2.ap[0][0], 16], [32, NG], [16, 2],
                                   [8, 2]])
                    src = bass.AP(psq.tensor, psq.offset,
                                  [[psq.ap[0][0], 16], [4, NG], [2, 2],
                                   [1, 2]])
                    nc.vector.tensor_copy(dst, src)
            idxw_flat = idxw2.rearrange("p k g c -> p (k g c)")
            for step in (16, 32, 64):
                nc.sync.dma_start(out=idxw_flat[step:2 * step, :],
                                  in_=idxw_flat[0:step, :])

            if debug:
                nc.sync.dma_start(out=dbg_wgt[:, :],
                                  in_=wg2.rearrange("p g k c -> p (g k c)"))
                nc.sync.dma_start(out=dbg_idxw[:, :],
                                  in_=idxw2.rearrange("p k g c -> p (k g c)"))

            nc.vector.tensor_copy(wg2b[:, :, :, :], wg2[:, :, :, :])

            keep = {"wg2", "wg2b", "idxw2", "outT", "ident", "identb", "woff_sb", "wmov_sb",
                    "boff_sb", "bias_sb", "xpad"}
            for nm in reversed(free_order):
                if nm not in keep:
                    free_fns.pop(nm)()

            # ---- P4: U_k = W_k @ x at all padded positions, per-k regions ----
            with (
                tc.tile_pool(name="utsb", bufs=2) as upool,
                tc.tile_pool(name="gat", bufs=3) as gpool,
                tc.tile_pool(name="gtb", bufs=1) as bpool,
                tc.tile_pool(name="red", bufs=1) as rpool,
                tc.tile_pool(name="osb", bufs=2) as obpool,
            ):
              for t in range(NT2):
                  psu = ps_ut.tile([128, NK * 256], F32, name="ps_ut_t")
                  for g in range(2):
                      xt = xpad[:, g, t * 128:(t + 1) * 128]
                      for nch in range(5):
                          sl = slice(nch * 512, min((nch + 1) * 512, NK * 256))
                          nc.tensor.matmul(psu[:, sl], xt, wmov_sb[:, g, sl],
                                           start=(g == 0), stop=(g == 1))
                  utsb = upool.tile([128, NK * 256], FP8, name="utsb_t")
                  nc.scalar.activation(utsb[:, :], psu[:, :],
                                       mybir.ActivationFunctionType.Copy)
                  # one DMA: utsb[p, k, o] -> ut_d[k, t*128+p, o]
                  dst = bass.AP(ut_d.tensor,
                                ut_d.offset + t * 128 * 256,
                                [[256, 128], [NQ * 256, NK], [1, 256]])
                  nc.sync.dma_start(out=dst, in_=utsb[:, :])

              # ---- P5: pair gathers + weighted reduce ----
              for grp in range(NG):
                  gt = gpool.tile([128, NK, 4, 512], FP8, name="gt_t")
                  for k in range(NK):
                      in_ap = bass.AP(ut_d.tensor,
                                      ut_d.offset + k * NQ * 256,
                                      [[256, NQ - 1], [1, 512]])
                      nc.gpsimd.dma_gather(
                          out_ap=gt[:, k, :, :],
                          in_ap=in_ap,
                          idxs_ap=idxw2[:, k, grp, :],
                          num_idxs=512, num_idxs_reg=512,
                          elem_size=512, elem_step=256)
                  # weight-multiply straight off the fp8 gather output
                  gtb = bpool.tile([128, NK, 4, 512], BF16, name="gtb_t")
                  gvin = bass.AP(gt.tensor, gt.offset,
                                 [gt.ap[0], [1024, 18], [512, 2], [256, 2],
                                  [1, 256]])
                  gv = bass.AP(gtb.tensor, gtb.offset,
                               [gtb.ap[0], [1024, 18], [512, 2], [256, 2],
                                [1, 256]])
                  wv = bass.AP(wg2b.tensor, wg2b.offset + grp * (NK * 8),
                               [wg2b.ap[0], [4, 18], [2, 2], [1, 2], [0, 256]])
                  nc.vector.tensor_tensor(gv, gvin, wv, op=mybir.AluOpType.mult)
                  # s1[kt, r, o] = xc0 + xc1
                  s1 = rpool.tile([128, 18, 2, 256], BF16, name="s1",
                                  tag="s1")
                  ga = bass.AP(gtb.tensor, gtb.offset,
                               [gtb.ap[0], [1024, 18], [512, 2], [1, 256]])
                  gb = bass.AP(gtb.tensor, gtb.offset + 256,
                               [gtb.ap[0], [1024, 18], [512, 2], [1, 256]])
                  nc.vector.tensor_add(s1[:, :, :, :], ga, gb)
                  # s2[kt, o] = r0 + r1
                  s2 = rpool.tile([128, 18, 256], BF16, name="s2", tag="s2")
                  nc.vector.tensor_add(s2[:, :, :], s1[:, :, 0, :],
                                       s1[:, :, 1, :])
                  # k-sum on PE: psum += I @ s2[k]  (f32 accumulation)
                  s2f = s2.rearrange("p a o -> p (a o)")
                  psk = ps_ut.tile([128, 512], F32, name="ps_ut_t")
                  for kk in range(NK):
                      nc.tensor.matmul(psk[:, :], identb[:, :],
                                       s2f[:, kk * 512:(kk + 1) * 512],
                                       start=(kk == 0), stop=(kk == NK - 1))
                  nc.scalar.activation(
                      outT.rearrange("p t o -> p (t o)")[:, grp * 512:(grp + 1) * 512],
                      psk[:, :], mybir.ActivationFunctionType.Copy)

              # ---- P6: transpose back, bias+sigmoid, store ----
              for t in range(NT):
                  for hh in range(2):
                      pso = ps_tr.tile([128, 128], F32, name="ps_tr_t")
                      nc.tensor.transpose(pso[:, :],
                                          outT[:, t, hh * 128:(hh + 1) * 128],
                                          ident[:, :])
                      osb = obpool.tile([128, 128], F32, name="osb_t")
                      nc.scalar.activation(osb[:, :], pso[:, :],
                                           mybir.ActivationFunctionType.Sigmoid,
                                           bias=bias_sb[:, hh:hh + 1], scale=1.0)
                      nc.sync.dma_start(
                          out=out_d[hh * 128:(hh + 1) * 128,
                                    t * 128:(t + 1) * 128],
                          in_=osb[:, :])
              if debug:
                  nc.sync.dma_start(out=dbg_outT[:, :],
                                    in_=outT.rearrange("p t o -> p (t o)"))

            for nm in reversed(free_order):
                if nm in free_fns:
                    free_fns.pop(nm)()

    nc.compile()
    return nc


def prepack(w_off, b_off, w, b):
    import ml_dtypes
    wofflhsT = np.zeros((2, NK, 128, 18), np.float32)
    for g in range(2):
        for t in range(NK):
            ty, tx = t // 3, t % 3
            wofflhsT[g, t] = w_off[:, g * 128:(g + 1) * 128, ty, tx].T
    wmov = np.zeros((2, 128, NK * 256), np.float32)
    for g in range(2):
        for k in range(NK):
            ky, kx = k // 3, k % 3
            wmov[g, :, k * 256:(k + 1) * 256] = w[:, g * 128:(g + 1) * 128, ky, kx].T
    p = np.arange(128)
    basey = (p[:, None] // 64 + 2 * np.arange(NT)[None, :]).astype(np.float32)
    basex = (p[:, None] % 64).astype(np.float32).copy()
    kk = np.arange(NK)
    sel = np.zeros((128, 8, 16), np.float32)
    for a in range(8):
        for bb in range(16):
            sel[16 * a + bb, a, bb] = 1.0
    return {
        "wofflhsT": np.ascontiguousarray(
            wofflhsT.transpose(2, 0, 1, 3)).astype(ml_dtypes.bfloat16),
        "wmov": np.ascontiguousarray(
            wmov.transpose(1, 0, 2)).astype(ml_dtypes.bfloat16),
        "boff": b_off.reshape(18, 1).astype(np.float32),
        "bias": np.stack([b[:128], b[128:]], axis=1).astype(np.float32).copy(),
        "ident": np.eye(128, dtype=np.float32),
        "basey": basey, "basex": basex,
        "kty": np.broadcast_to((kk // 3 - 1).astype(np.float32), (128, NK)).copy(),
        "ktx": np.broadcast_to((kk % 3 - 1).astype(np.float32), (128, NK)).copy(),
        "sel": sel,
    }


def make_in_maps(Fstagei, w_off, b_off, w, b):
    shared = prepack(np.asarray(w_off), np.asarray(b_off), np.asarray(w),
                     np.asarray(b))
    in_maps = []
    for i in range(B):
        m = dict(shared)
        m["x"] = np.ascontiguousarray(
            np.asarray(Fstagei[i]).reshape(C, HW).astype(np.float32))
        in_maps.append(m)
    return in_maps


def kernel(Fstagei, w_off, b_off, w, b):
    if "nc" not in _CACHE:
        _CACHE["nc"] = build_nc()
    nc = _CACHE["nc"]
    in_maps = make_in_maps(Fstagei, w_off, b_off, w, b)
    res = run_bass_kernel_spmd(nc, in_maps, core_ids=list(range(B)), trace=False)
    out = np.stack([np.asarray(res.results[i]["out"]).reshape(C, H, W)
                    for i in range(B)])
    return out.astype(np.float32)

